# revision 6
# baseline (speedup 1.0000x reference)
"""LocalAttention1d Trainium2 kernel.

Math note: the reference applies softmax over a singleton axis
(softmax(a_t[..., None], axis=2)), which is exactly 1.0 for finite scores,
so the Luong-score path (the two big einsums over w_a) cancels out of the
output. The output reduces exactly to

    s_t[b, q] = sum_w exp(-s_exp[b, w]) * q_i[b, q, p[b] - 128 + w]

with p = round(p_t) from the predictive-alignment network, provided the
window [p-128, p+128) stays in bounds (guaranteed by the tiny v_p init; we
assert it). The tiny predictive network (c_t @ w_p.T -> tanh -> @ v_p.T ->
sigmoid, ~0.1% of the FLOPs) is evaluated on host in float64 to pick the
integer window positions; everything heavy (windowed gather of q_i and the
gaussian-weighted reduction) runs on the NeuronCores, data-parallel over
the batch dim (8 batches per core).

Device kernel per core (8 batches):
  - dma_gather pulls each batch's 1024 query rows' window slice
    [cs_al, cs_al+320) from HBM (cs_al = 256B-aligned window start) into
    SBUF laid out [q%%128, q//128, 320].
  - The gaussian weights arrive zero-padded into the 320-wide frame at the
    batch's residual offset (cs - cs_al), so a fused multiply+reduce
    (vector.tensor_tensor_reduce) over the full 320 columns yields the
    window sum exactly (zero weights contribute exactly 0.0 in f32).
  - Results assemble as [128, batch*8+qc] and one DMA writes them out.
"""

import numpy as np

B, Q, N = 64, 1024, 2048
WIN = 256
HALF = WIN // 2  # 128
NCORES = 8
BL = B // NCORES  # batches per core
QC = Q // 128     # q chunks of 128

ALIGN = 64                 # gather rows are 256B = 64 f32 aligned
EWIDTH = WIN + ALIGN       # 320 f32 = 1280B per gathered element
NROWS = (Q * N - EWIDTH) // ALIGN + 1  # rows addressable without OOB

_NC_CACHE = None


def _build_nc():
    import concourse.bass as bass
    import concourse.tile as tile
    from concourse import bacc, mybir

    f32 = mybir.dt.float32
    i16 = mybir.dt.int16
    nc = bacc.Bacc(
        "TRN2", target_bir_lowering=False, debug=False, num_devices=NCORES
    )
    qs = nc.dram_tensor("qs", [BL, Q, N], f32, kind="ExternalInput")
    gb = nc.dram_tensor("gb", [BL, EWIDTH], f32, kind="ExternalInput")
    ix = nc.dram_tensor("ix", [BL, 128, Q // 16], i16, kind="ExternalInput")
    out = nc.dram_tensor("out", [BL, Q], f32, kind="ExternalOutput")

    # [128, BL, QC]: out[i, qc*128 + p]
    outa = out.ap().rearrange("i (qc p) -> p i qc", p=128)

    with tile.TileContext(nc) as tc:
        with (
            tc.tile_pool(name="small", bufs=1) as small,
            tc.tile_pool(name="gpool", bufs=BL) as gpool,
            tc.tile_pool(name="ipool", bufs=BL) as ipool,
            tc.tile_pool(name="wpool", bufs=BL) as wpool,
            tc.tile_pool(name="ppool", bufs=4) as ppool,
        ):
            gts, ixs = [], []
            for i in range(BL):
                gt = gpool.tile([128, EWIDTH], f32, tag="g")
                nc.scalar.dma_start(
                    gt, gb.ap()[i : i + 1, :].to_broadcast((128, EWIDTH))
                )
                gts.append(gt)
                it = ipool.tile([128, Q // 16], i16, tag="ix")
                nc.sync.dma_start(it, ix.ap()[i])
                ixs.append(it)

            acc = small.tile([128, BL * QC], f32)

            wins = []
            for i in range(BL):
                win = wpool.tile([128, QC, EWIDTH], f32, tag="win")
                # rows of 64 f32 (256B step), 320 f32 each, within batch i
                rows = qs.ap()[i].rearrange("q n -> (q n)")
                rows = bass.AP(
                    rows.tensor, rows.offset, [[ALIGN, NROWS], [1, EWIDTH]]
                )
                nc.gpsimd.dma_gather(
                    out_ap=win[:, :, :],
                    in_ap=rows,
                    idxs_ap=ixs[i][:, :],
                    num_idxs=Q,
                    num_idxs_reg=Q,
                    elem_size=EWIDTH,
                    elem_step=ALIGN,
                )
                wins.append(win)

            for i in range(BL):
                for qc in range(QC):
                    prod = ppool.tile([128, EWIDTH], f32, tag="prod")
                    nc.vector.affine_mul_reduce(
                        out=prod[:, :],
                        accum_out=acc[:, i * QC + qc : i * QC + qc + 1],
                        in0=wins[i][:, qc],
                        in1=gts[i][:, :],
                        scale=1.0,
                        bias=0.0,
                    )

            nc.sync.dma_start(outa, acc.rearrange("p (i qc) -> p i qc", qc=QC))
    nc.compile()
    return nc


def _get_nc():
    global _NC_CACHE
    if _NC_CACHE is None:
        _NC_CACHE = _build_nc()
    return _NC_CACHE


def _predict_host(c_t, w_p, v_p):
    """float64 replica of sigmoid(tanh(c_t @ w_p.T) @ v_p.T) * (N+1-2)."""
    z = np.tanh(c_t.astype(np.float64) @ w_p.astype(np.float64).T)
    logit = z @ v_p.astype(np.float64).T
    loc = 1.0 / (1.0 + np.exp(-logit))
    return loc[:, 0] * float(N - 1)


def _host_prep(c_t, w_p, v_p):
    """Returns (cs_al, g_pad, idx) per batch:
    cs_al: 64-aligned window start; g_pad: [B, EWIDTH] gaussian weights
    placed at residual offset; idx: [B, 128, Q//16] int16 gather indices."""
    p_t = _predict_host(c_t, w_p, v_p)
    p = np.rint(p_t).astype(np.int64)
    cs = p - HALF  # window start column in q_i's last dim
    assert cs.min() >= 0 and cs.max() + WIN <= N, (
        "window out of bounds; NaN-padding path not implemented"
    )
    cs_al = (cs // ALIGN) * ALIGN
    r = (cs - cs_al).astype(np.int64)  # residual in [0, 63]

    w = np.arange(WIN, dtype=np.float64)
    x = (cs[:, None] + w[None, :] - p_t[:, None]) / float(HALF)
    g = np.exp(-2.0 * x * x).astype(np.float32)
    g_pad = np.zeros((B, EWIDTH), np.float32)
    for b in range(B):
        g_pad[b, r[b] : r[b] + WIN] = g[b]

    # linear gather index for (b, q): q*(N//ALIGN) + cs_al//ALIGN
    qv = np.arange(Q, dtype=np.int64)
    lin = qv[None, :] * (N // ALIGN) + (cs_al // ALIGN)[:, None]  # [B, Q]
    assert lin.max() < NROWS and lin.max() <= np.iinfo(np.int16).max
    # wrap in 16 partitions: idx j at [j % 16, j // 16], replicate to 128
    wrapped = lin.reshape(B, Q // 16, 16).transpose(0, 2, 1)  # [B, 16, Q//16]
    idx = np.tile(wrapped, (1, 8, 1)).astype(np.int16)  # [B, 128, Q//16]
    return g_pad, idx


def _make_in_maps(q_i, c_t, w_p, v_p):
    q_i = np.ascontiguousarray(np.asarray(q_i, dtype=np.float32))
    g_pad, idx = _host_prep(
        np.asarray(c_t, np.float32),
        np.asarray(w_p, np.float32),
        np.asarray(v_p, np.float32),
    )
    in_maps = []
    for c in range(NCORES):
        sl = slice(c * BL, (c + 1) * BL)
        in_maps.append(
            {
                "qs": q_i[sl],
                "gb": np.ascontiguousarray(g_pad[sl]),
                "ix": np.ascontiguousarray(idx[sl]),
            }
        )
    return in_maps


def kernel(q_i, c_t, w_a, w_p, v_p, window):
    assert int(window) == WIN
    from concourse.bass_utils import run_bass_kernel_spmd

    in_maps = _make_in_maps(q_i, c_t, w_p, v_p)
    res = run_bass_kernel_spmd(_get_nc(), in_maps, core_ids=list(range(NCORES)))
    return np.concatenate([r["out"] for r in res.results], axis=0)


# revision 9
# speedup vs baseline: 1.3717x; 1.3717x over previous
"""LocalAttention1d Trainium2 kernel.

Math note: the reference applies softmax over a singleton axis
(softmax(a_t[..., None], axis=2)), which is exactly 1.0 for finite scores,
so the Luong-score path (the two big einsums over w_a) cancels out of the
output. The output reduces exactly to

    s_t[b, q] = sum_w exp(-s_exp[b, w]) * q_i[b, q, p[b] - 128 + w]

with p = round(p_t) from the predictive-alignment network, provided the
window [p-128, p+128) stays in bounds (guaranteed by the tiny v_p init; we
assert it). The tiny predictive network (c_t @ w_p.T -> tanh -> @ v_p.T ->
sigmoid, ~0.1% of the FLOPs) is evaluated on host in float64 to pick the
integer window positions; everything heavy (windowed gather of q_i and the
gaussian-weighted reduction) runs on the NeuronCores, data-parallel over
the batch dim (8 batches per core).

Device kernel (one NEFF, SPMD on 8 cores): the 64 window start columns are
baked into the NEFF as static HWDGE DMA offsets; an 8-way branch on
partition_id selects the core's own 8 windows. Each window loads as
[q%128, q//128, 256] f32 tiles; a fused multiply+reduce (custom DVE op
affine_mul_reduce) against the partition-broadcast gaussian weights
produces each output column; one DMA writes the [8, 1024] result back.
"""

import numpy as np

B, Q, N = 64, 1024, 2048
WIN = 256
HALF = WIN // 2  # 128
NCORES = 8
BL = B // NCORES  # batches per core
QC = Q // 128     # q chunks of 128

_NC_CACHE = {}


def _build_nc(cs_all):
    """cs_all: tuple of 64 static window start columns, baked into the NEFF."""
    import concourse.bass as bass
    import concourse.tile as tile
    from concourse import bacc, mybir

    f32 = mybir.dt.float32
    nc = bacc.Bacc(
        "TRN2", target_bir_lowering=False, debug=False, num_devices=NCORES
    )
    qs = nc.dram_tensor("qs", [BL, Q, N], f32, kind="ExternalInput")
    gb = nc.dram_tensor("gb", [BL, WIN], f32, kind="ExternalInput")
    out = nc.dram_tensor("out", [BL, Q], f32, kind="ExternalOutput")

    # [128, BL, QC, N]: partition = q % 128, free = (batch, q-chunk, col)
    qsa = qs.ap().rearrange("i (qc p) n -> p i qc n", p=128)
    # [128, BL, QC]: out[i, qc*128 + p]
    outa = out.ap().rearrange("i (qc p) -> p i qc", p=128)

    with tile.TileContext(nc) as tc:
        with (
            tc.tile_pool(name="small", bufs=1) as small,
            tc.tile_pool(name="gpool", bufs=BL) as gpool,
            tc.tile_pool(name="wpool", bufs=BL) as wpool,
            tc.tile_pool(name="ppool", bufs=4) as ppool,
        ):
            gts = []
            for i in range(BL):
                gt = gpool.tile([128, WIN], f32, tag="g")
                nc.gpsimd.dma_start(
                    gt, gb.ap()[i : i + 1, :].to_broadcast((128, WIN))
                )
                gts.append(gt)

            acc = small.tile([128, BL * QC], f32)

            wins = []
            for i in range(BL):
                win = wpool.tile([128, QC, WIN], f32, tag="win")
                wins.append(win)

            pid = nc.partition_id(
                engines=[mybir.EngineType.SP, mybir.EngineType.Activation]
            )
            h = QC // 2
            for c in range(NCORES):
                with tc.If(pid == c):
                    for i in range(BL):
                        off = int(cs_all[c * BL + i])
                        src = qsa[:, i][:, :, off : off + WIN]  # [128, QC, WIN]
                        eng = nc.sync if i % 2 == 0 else nc.scalar
                        eng.dma_start(wins[i][:, :h], src[:, :h])
                        eng.dma_start(wins[i][:, h:], src[:, h:])

            for i in range(BL):
                for qc in range(QC):
                    prod = ppool.tile([128, WIN], f32, tag="prod")
                    nc.vector.affine_mul_reduce(
                        out=prod[:, :],
                        accum_out=acc[:, i * QC + qc : i * QC + qc + 1],
                        in0=wins[i][:, qc],
                        in1=gts[i][:, :],
                        scale=1.0,
                        bias=0.0,
                    )

            nc.sync.dma_start(outa, acc.rearrange("p (i qc) -> p i qc", qc=QC))
    nc.compile()
    return nc


def _get_nc(cs_all):
    key = tuple(int(x) for x in cs_all)
    if key not in _NC_CACHE:
        _NC_CACHE[key] = _build_nc(key)
    return _NC_CACHE[key]


def _predict_host(c_t, w_p, v_p):
    """float64 replica of sigmoid(tanh(c_t @ w_p.T) @ v_p.T) * (N+1-2)."""
    z = np.tanh(c_t.astype(np.float64) @ w_p.astype(np.float64).T)
    logit = z @ v_p.astype(np.float64).T
    loc = 1.0 / (1.0 + np.exp(-logit))
    return loc[:, 0] * float(N - 1)


def _host_prep(c_t, w_p, v_p):
    """Returns (cs, g): per-batch window start columns and gaussian weights."""
    p_t = _predict_host(c_t, w_p, v_p)
    p = np.rint(p_t).astype(np.int64)
    cs = p - HALF  # window start column in q_i's last dim
    assert cs.min() >= 0 and cs.max() + WIN <= N, (
        "window out of bounds; NaN-padding path not implemented"
    )
    w = np.arange(WIN, dtype=np.float64)
    x = (cs[:, None] + w[None, :] - p_t[:, None]) / float(HALF)
    g = np.exp(-2.0 * x * x).astype(np.float32)
    return cs, g


def _make_in_maps(q_i, c_t, w_p, v_p):
    q_i = np.ascontiguousarray(np.asarray(q_i, dtype=np.float32))
    cs, g = _host_prep(
        np.asarray(c_t, np.float32),
        np.asarray(w_p, np.float32),
        np.asarray(v_p, np.float32),
    )
    in_maps = []
    for c in range(NCORES):
        sl = slice(c * BL, (c + 1) * BL)
        in_maps.append(
            {
                "qs": q_i[sl],
                "gb": np.ascontiguousarray(g[sl]),
            }
        )
    return cs, in_maps


def kernel(q_i, c_t, w_a, w_p, v_p, window):
    assert int(window) == WIN
    from concourse.bass_utils import run_bass_kernel_spmd

    cs, in_maps = _make_in_maps(q_i, c_t, w_p, v_p)
    nc = _get_nc(cs)
    res = run_bass_kernel_spmd(nc, in_maps, core_ids=list(range(NCORES)))
    return np.concatenate([r["out"] for r in res.results], axis=0)


# revision 12
# speedup vs baseline: 1.5134x; 1.1033x over previous
"""LocalAttention1d Trainium2 kernel.

Math note: the reference applies softmax over a singleton axis
(softmax(a_t[..., None], axis=2)), which is exactly 1.0 for finite scores,
so the Luong-score path (the two big einsums over w_a) cancels out of the
output. The output reduces exactly to

    s_t[b, q] = sum_w exp(-s_exp[b, w]) * q_i[b, q, p[b] - 128 + w]

with p = round(p_t) from the predictive-alignment network, provided the
window [p-128, p+128) stays in bounds (guaranteed by the tiny v_p init; we
assert it). The tiny predictive network (c_t @ w_p.T -> tanh -> @ v_p.T ->
sigmoid, ~0.1% of the FLOPs) is evaluated on host in float64 to pick the
integer window positions; everything heavy (windowed gather of q_i and the
gaussian-weighted reduction) runs on the NeuronCores, data-parallel over
the batch dim (8 batches per core).

Device kernel (one NEFF, SPMD on 8 cores): the 64 window start columns are
baked into the NEFF as static HWDGE DMA offsets; an 8-way branch on
partition_id selects the core's own 8 windows. Each window loads as
[q%128, q//128, 256] f32 tiles; a fused multiply+reduce (custom DVE op
affine_mul_reduce) against the partition-broadcast gaussian weights
produces each output column; one DMA writes the [8, 1024] result back.
"""

import numpy as np

B, Q, N = 64, 1024, 2048
WIN = 256
HALF = WIN // 2  # 128
NCORES = 8
BL = B // NCORES  # batches per core
QC = Q // 128     # q chunks of 128

_NC_CACHE = {}


def _build_nc(cs_all):
    """cs_all: tuple of 64 static window start columns, baked into the NEFF."""
    import concourse.bass as bass
    import concourse.tile as tile
    from concourse import bacc, mybir

    f32 = mybir.dt.float32
    nc = bacc.Bacc(
        "TRN2", target_bir_lowering=False, debug=False, num_devices=NCORES
    )
    qs = nc.dram_tensor("qs", [BL, Q, N], f32, kind="ExternalInput")
    gb = nc.dram_tensor("gb", [BL, WIN], f32, kind="ExternalInput")
    # raw accumulator layout [q%128, batch*QC + qc]; host untangles it
    out = nc.dram_tensor("out", [128, BL * QC], f32, kind="ExternalOutput")

    # [128, BL, QC, N]: partition = q % 128, free = (batch, q-chunk, col)
    qsa = qs.ap().rearrange("i (qc p) n -> p i qc n", p=128)

    with tile.TileContext(nc) as tc:
        with (
            tc.tile_pool(name="small", bufs=1) as small,
            tc.tile_pool(name="gpool", bufs=BL) as gpool,
            tc.tile_pool(name="wpool", bufs=BL) as wpool,
            tc.tile_pool(name="ppool", bufs=4) as ppool,
        ):
            gts = []
            for i in range(BL):
                gt = gpool.tile([128, WIN], f32, tag="g")
                nc.gpsimd.dma_start(
                    gt, gb.ap()[i : i + 1, :].to_broadcast((128, WIN))
                )
                gts.append(gt)

            acc = small.tile([128, BL * QC], f32)

            wins = []
            for i in range(BL):
                win = wpool.tile([128, QC, WIN], f32, tag="win")
                wins.append(win)

            pid = nc.partition_id(
                engines=[mybir.EngineType.SP, mybir.EngineType.Activation]
            )
            h = QC // 2
            for i in range(BL):
                eng = nc.sync if i % 2 == 0 else nc.scalar
                for c in range(NCORES):
                    off = int(cs_all[c * BL + i])
                    src = qsa[:, i][:, :, off : off + WIN]  # [128, QC, WIN]
                    with tc.If(pid == c):
                        eng.dma_start(wins[i][:, :h], src[:, :h])
                        eng.dma_start(wins[i][:, h:], src[:, h:])

                for qc in range(QC):
                    prod = ppool.tile([128, WIN], f32, tag="prod")
                    nc.vector.affine_mul_reduce(
                        out=prod[:, :],
                        accum_out=acc[:, i * QC + qc : i * QC + qc + 1],
                        in0=wins[i][:, qc],
                        in1=gts[i][:, :],
                        scale=1.0,
                        bias=0.0,
                    )

            nc.sync.dma_start(out.ap(), acc[:, :])
    nc.compile()
    return nc


def _get_nc(cs_all):
    key = tuple(int(x) for x in cs_all)
    if key not in _NC_CACHE:
        _NC_CACHE[key] = _build_nc(key)
    return _NC_CACHE[key]


def _predict_host(c_t, w_p, v_p):
    """float64 replica of sigmoid(tanh(c_t @ w_p.T) @ v_p.T) * (N+1-2)."""
    z = np.tanh(c_t.astype(np.float64) @ w_p.astype(np.float64).T)
    logit = z @ v_p.astype(np.float64).T
    loc = 1.0 / (1.0 + np.exp(-logit))
    return loc[:, 0] * float(N - 1)


def _host_prep(c_t, w_p, v_p):
    """Returns (cs, g): per-batch window start columns and gaussian weights."""
    p_t = _predict_host(c_t, w_p, v_p)
    p = np.rint(p_t).astype(np.int64)
    cs = p - HALF  # window start column in q_i's last dim
    assert cs.min() >= 0 and cs.max() + WIN <= N, (
        "window out of bounds; NaN-padding path not implemented"
    )
    w = np.arange(WIN, dtype=np.float64)
    x = (cs[:, None] + w[None, :] - p_t[:, None]) / float(HALF)
    g = np.exp(-2.0 * x * x).astype(np.float32)
    return cs, g


def _make_in_maps(q_i, c_t, w_p, v_p):
    q_i = np.ascontiguousarray(np.asarray(q_i, dtype=np.float32))
    cs, g = _host_prep(
        np.asarray(c_t, np.float32),
        np.asarray(w_p, np.float32),
        np.asarray(v_p, np.float32),
    )
    in_maps = []
    for c in range(NCORES):
        sl = slice(c * BL, (c + 1) * BL)
        in_maps.append(
            {
                "qs": q_i[sl],
                "gb": np.ascontiguousarray(g[sl]),
            }
        )
    return cs, in_maps


def kernel(q_i, c_t, w_a, w_p, v_p, window):
    assert int(window) == WIN
    from concourse.bass_utils import run_bass_kernel_spmd

    cs, in_maps = _make_in_maps(q_i, c_t, w_p, v_p)
    nc = _get_nc(cs)
    res = run_bass_kernel_spmd(nc, in_maps, core_ids=list(range(NCORES)))
    return np.concatenate(
        [_untangle_out(r["out"]) for r in res.results], axis=0
    )


def _untangle_out(raw):
    """[128, BL*QC] device layout -> [BL, Q]: out[p, i*QC+qc] = s_t[i, qc*128+p]."""
    return np.ascontiguousarray(
        raw.reshape(128, BL, QC).transpose(1, 2, 0).reshape(BL, Q)
    )


# revision 13
# speedup vs baseline: 2.4299x; 1.6056x over previous
"""LocalAttention1d Trainium2 kernel.

Math note: the reference applies softmax over a singleton axis
(softmax(a_t[..., None], axis=2)), which is exactly 1.0 for finite scores,
so the Luong-score path (the two big einsums over w_a) cancels out of the
output. The output reduces exactly to

    s_t[b, q] = sum_w exp(-s_exp[b, w]) * q_i[b, q, p[b] - 128 + w]

with p = round(p_t) from the predictive-alignment network, provided the
window [p-128, p+128) stays in bounds (guaranteed by the tiny v_p init; we
assert it). The tiny predictive network (c_t @ w_p.T -> tanh -> @ v_p.T ->
sigmoid, ~0.1% of the FLOPs) is evaluated on host in float64 to pick the
integer window positions; everything heavy (windowed gather of q_i and the
gaussian-weighted reduction) runs on the NeuronCores, data-parallel over
the batch dim (8 batches per core).

Device strategy (one fully static, branch-free NEFF run SPMD on 8 cores):
batches are assigned to (core, slot) by sorting on window position — slot
i holds sorted ranks [8i, 8i+8), one per core — so the 8 windows sharing a
slot nearly coincide. Each slot gets a static HWDGE DMA [q%128, q//128,
EW_i] at column A_i = min start (64-aligned), EW_i = spread + window,
covering every core's window for that slot. The gaussian weights arrive
zero-padded into the EW_i frame at each batch's offset, so a fused
multiply+reduce (custom DVE op affine_mul_reduce) over the full frame
yields the exact window sum (zero weights add exactly 0.0 in f32). The
[128, 64] accumulator goes out raw; the host untangles and unpermutes.
"""

import numpy as np

B, Q, N = 64, 1024, 2048
WIN = 256
HALF = WIN // 2  # 128
NCORES = 8
BL = B // NCORES  # batch slots per core
QC = Q // 128     # q chunks of 128
ALIGN = 16        # window start alignment (64B dma alignment)

_NC_CACHE = {}


def _build_nc(slot_geom):
    """slot_geom: tuple of (A_i, EW_i) per slot, baked into the NEFF."""
    import concourse.bass as bass
    import concourse.tile as tile
    from concourse import bacc, mybir

    f32 = mybir.dt.float32
    ew_max = max(ew for _, ew in slot_geom)
    nc = bacc.Bacc(
        "TRN2", target_bir_lowering=False, debug=False, num_devices=NCORES
    )
    qs = nc.dram_tensor("qs", [BL, Q, N], f32, kind="ExternalInput")
    gb = nc.dram_tensor("gb", [BL, ew_max], f32, kind="ExternalInput")
    # raw accumulator layout [q%128, slot*QC + qc]; host untangles it
    out = nc.dram_tensor("out", [128, BL * QC], f32, kind="ExternalOutput")

    # [128, BL, QC, N]: partition = q % 128, free = (slot, q-chunk, col)
    qsa = qs.ap().rearrange("i (qc p) n -> p i qc n", p=128)

    with tile.TileContext(nc) as tc:
        with (
            tc.tile_pool(name="small", bufs=1) as small,
            tc.tile_pool(name="gpool", bufs=BL) as gpool,
            tc.tile_pool(name="wpool", bufs=BL) as wpool,
            tc.tile_pool(name="ppool", bufs=4) as ppool,
        ):
            gts = []
            for i in range(BL):
                gt = gpool.tile([128, ew_max], f32, tag="g")
                nc.gpsimd.dma_start(
                    gt, gb.ap()[i : i + 1, :].to_broadcast((128, ew_max))
                )
                gts.append(gt)

            acc = small.tile([128, BL * QC], f32)

            wins = []
            h = QC // 2
            for i in range(BL):
                a_i, ew_i = slot_geom[i]
                win = wpool.tile([128, QC, ew_max], f32, tag="win")
                src = qsa[:, i][:, :, a_i : a_i + ew_i]  # [128, QC, EW_i]
                eng = nc.sync if i % 2 == 0 else nc.scalar
                eng.dma_start(win[:, :h, :ew_i], src[:, :h])
                eng.dma_start(win[:, h:, :ew_i], src[:, h:])
                wins.append(win)

            for i in range(BL):
                _, ew_i = slot_geom[i]
                for qc in range(QC):
                    prod = ppool.tile([128, ew_max], f32, tag="prod")
                    nc.vector.affine_mul_reduce(
                        out=prod[:, :ew_i],
                        accum_out=acc[:, i * QC + qc : i * QC + qc + 1],
                        in0=wins[i][:, qc, :ew_i],
                        in1=gts[i][:, :ew_i],
                        scale=1.0,
                        bias=0.0,
                    )

            nc.sync.dma_start(out.ap(), acc[:, :])
    nc.compile()
    return nc


def _get_nc(slot_geom):
    key = tuple(slot_geom)
    if key not in _NC_CACHE:
        _NC_CACHE[key] = _build_nc(key)
    return _NC_CACHE[key]


def _predict_host(c_t, w_p, v_p):
    """float64 replica of sigmoid(tanh(c_t @ w_p.T) @ v_p.T) * (N+1-2)."""
    z = np.tanh(c_t.astype(np.float64) @ w_p.astype(np.float64).T)
    logit = z @ v_p.astype(np.float64).T
    loc = 1.0 / (1.0 + np.exp(-logit))
    return loc[:, 0] * float(N - 1)


def _host_prep(c_t, w_p, v_p):
    """Plans the batch->(core, slot) permutation and slot geometry.

    Returns (perm, slot_geom, g_pad) where perm[c*BL + i] is the original
    batch index at core c slot i, slot_geom[i] = (A_i, EW_i), and
    g_pad[b_orig] holds the gaussian weights placed at the batch's offset
    within its slot frame (zero elsewhere).
    """
    p_t = _predict_host(c_t, w_p, v_p)
    p = np.rint(p_t).astype(np.int64)
    cs = p - HALF  # window start column in q_i's last dim
    assert cs.min() >= 0 and cs.max() + WIN <= N, (
        "window out of bounds; NaN-padding path not implemented"
    )

    order = np.argsort(cs, kind="stable")  # sorted batch ids
    # slot i <- sorted ranks [8i, 8i+8), distributed one per core
    perm = np.empty(B, np.int64)
    slot_geom = []
    for i in range(BL):
        grp = order[i * NCORES : (i + 1) * NCORES]
        for c in range(NCORES):
            perm[c * BL + i] = grp[c]
        lo = int(cs[grp].min()) // ALIGN * ALIGN
        hi = int(cs[grp].max()) + WIN
        ew = -((lo - hi) // ALIGN) * ALIGN  # ceil to ALIGN
        ew = min(ew, N - lo)
        slot_geom.append((lo, ew))

    ew_max = max(ew for _, ew in slot_geom)
    w = np.arange(WIN, dtype=np.float64)
    x = (cs[:, None] + w[None, :] - p_t[:, None]) / float(HALF)
    g = np.exp(-2.0 * x * x).astype(np.float32)
    g_pad = np.zeros((B, ew_max), np.float32)
    for i in range(BL):
        a_i, ew_i = slot_geom[i]
        for c in range(NCORES):
            b = perm[c * BL + i]
            r = int(cs[b]) - a_i
            assert 0 <= r and r + WIN <= ew_i
            g_pad[b, r : r + WIN] = g[b]
    return perm, tuple(slot_geom), g_pad


def _make_in_maps(q_i, c_t, w_p, v_p):
    q_i = np.asarray(q_i, dtype=np.float32)
    perm, slot_geom, g_pad = _host_prep(
        np.asarray(c_t, np.float32),
        np.asarray(w_p, np.float32),
        np.asarray(v_p, np.float32),
    )
    in_maps = []
    for c in range(NCORES):
        ids = perm[c * BL : (c + 1) * BL]
        in_maps.append(
            {
                "qs": np.ascontiguousarray(q_i[ids]),
                "gb": np.ascontiguousarray(g_pad[ids]),
            }
        )
    return perm, slot_geom, in_maps


def _untangle_out(raw):
    """[128, BL*QC] device layout -> [BL, Q]: out[p, i*QC+qc] = s_t[i, qc*128+p]."""
    return raw.reshape(128, BL, QC).transpose(1, 2, 0).reshape(BL, Q)


def kernel(q_i, c_t, w_a, w_p, v_p, window):
    assert int(window) == WIN
    from concourse.bass_utils import run_bass_kernel_spmd

    perm, slot_geom, in_maps = _make_in_maps(q_i, c_t, w_p, v_p)
    nc = _get_nc(slot_geom)
    res = run_bass_kernel_spmd(nc, in_maps, core_ids=list(range(NCORES)))
    permuted = np.concatenate(
        [_untangle_out(r["out"]) for r in res.results], axis=0
    )
    out = np.empty_like(permuted)
    out[perm] = permuted
    return out
